# revision 18
# baseline (speedup 1.0000x reference)
"""Trainium2 Bass kernel for a 3-layer GCN (JKNet, mode='cat') — 8-core SPMD.

Strategy (dst-sharded graph parallelism):
  - Nodes are partitioned across 8 cores (6250 each, padded to 6272 = 49*128).
    Each core owns all edges whose destination lands in its range.
  - Per layer: each core computes its slice of h @ W, an AllGather builds the
    full 50176x64 f32 gather table in DRAM, then the core gathers hW[src] for
    its edges with GPSIMD dma_gather (4 SWDGE queues), scales by edge weight
    on DVE and accumulates into an SBUF accumulator using a "rounds" schedule:
    nodes are degree-sorted so round s covers a dense prefix of node slots,
    making the accumulate a plain strided DVE add (no scatter anywhere).
  - dma_gather indices are int16, so the table is split in two halves
    (rows < 25088 / >= 25088, i.e. src cores 0-3 vs 4-7). Each half gets its
    own degree-sort permutation and accumulator; the second accumulator is
    reconciled through a DRAM bounce + local permute-gather.
  - Bias + ReLU on DVE/ACT, PE transpose produces h^T for the next layer's
    matmul and for the final JumpingKnowledge concat matmul.

Self-contained: hardcodes the problem geometry (N=50000, E=800000, 128->64,
3 layers, out 40) but computes all data-dependent schedules from the inputs.
"""

import sys

sys.path.insert(0, "/opt/trn_rl_repo")

import numpy as np

N = 50000
E = 800000
IN_DIM = 128
HID = 64
OUT_DIM = 40
M = 8               # cores
NPC = N // M        # 6250 nodes per core
SLOTS = 49          # ceil(6250/128)
SLICE = SLOTS * 128  # 6272 padded rows per core slice
TABLE_ROWS = M * SLICE  # 50176
HALF = 4 * SLICE    # 25088 (int16-safe boundary; src cores 0-3 vs 4-7)
CMAX = 4096         # max indices per dma_gather instruction
NQ = 4              # SWDGE queues


def _wrap16(a):
    """Flat [L] -> [128, L//16] int16, index j at partition j%16, slot j//16,
    replicated across the 8 GPSIMD core groups."""
    L = a.shape[0]
    return np.tile(a.reshape(L // 16, 16).T, (8, 1)).astype(np.int16)


def _wrap128(a):
    """Flat [L] -> [128, L//128], position j at partition j%128, slot j//128."""
    L = a.shape[0]
    return np.ascontiguousarray(a.reshape(L // 128, 128).T)


def _rowof(q):
    """acc position q -> wrapped DRAM row index (partition-major layout)."""
    return (q % 128) * SLOTS + q // 128


def _ranks_within(p):
    """For int array p, rank of each element among equal values (stable)."""
    order = np.argsort(p, kind="stable")
    ps = p[order]
    starts = np.r_[0, np.nonzero(np.diff(ps))[0] + 1]
    counts = np.diff(np.r_[starts, len(ps)])
    r_sorted = np.arange(len(ps)) - np.repeat(starts, counts)
    r = np.empty_like(r_sorted)
    r[order] = r_sorted
    return r


def _build_system(pos_of_dst, table_row_of_src, ew, max_rounds_widths):
    """Build flat (idx, ew) arrays for one round-system of one core.

    pos_of_dst: per-edge acc position of the destination node (degree-sorted).
    table_row_of_src: per-edge gather index (already half-relative).
    max_rounds_widths: global per-round padded widths W_s (list, multiple of 128).
    Returns (idx_flat int64, ew_flat f32).
    """
    roundoff = np.r_[0, np.cumsum(max_rounds_widths)]
    L = int(roundoff[-1])
    idx_flat = np.zeros(L, np.int64)
    ew_flat = np.zeros(L, np.float32)
    r = _ranks_within(pos_of_dst)
    flatpos = roundoff[r] + pos_of_dst
    idx_flat[flatpos] = table_row_of_src
    ew_flat[flatpos] = ew
    return idx_flat, ew_flat


def _plan_rounds(deg_by_core):
    """deg_by_core: [M, NPC] degree of each node (in its own sort order, desc).
    Returns list of global padded round widths (each a multiple of 128)."""
    smax = int(max(d[0] for d in deg_by_core)) if len(deg_by_core) else 0
    widths = []
    for s in range(smax):
        n_s = max(int((d > s).sum()) for d in deg_by_core)
        if n_s == 0:
            break
        widths.append(((n_s + 127) // 128) * 128)
    return widths


def _chunk_plan(widths, total_pad128):
    """Split flat [0, L) (concatenated padded rounds) into gather chunks of at
    most CMAX (multiples of 128), with per-chunk accumulate segments.
    Returns list of (off, w, [(msg_slot0, acc_slot0, nslots), ...])."""
    roundoff = np.r_[0, np.cumsum(widths)].astype(np.int64)
    L = int(roundoff[-1])
    assert L == total_pad128
    chunks = []
    off = 0
    while off < L:
        w = min(CMAX, L - off)
        # segments: intersect [off, off+w) with rounds
        segs = []
        s = int(np.searchsorted(roundoff, off, side="right")) - 1
        a = off
        while a < off + w:
            b = min(off + w, int(roundoff[s + 1]))
            segs.append(((a - off) // 128, int(a - roundoff[s]) // 128, (b - a) // 128))
            a = b
            s += 1
        chunks.append((off, w, segs))
        off += w
    return chunks


def _prep(x, edge_index, edge_weight):
    """All host-side index prep. Returns (plan dict, per-core input maps)."""
    src = np.asarray(edge_index[0], dtype=np.int64)
    dst = np.asarray(edge_index[1], dtype=np.int64)
    ew = np.asarray(edge_weight, dtype=np.float32)
    x = np.asarray(x, dtype=np.float32)

    dcore = dst // NPC
    dloc = dst - dcore * NPC
    score = src // NPC
    is_a = score <= 3

    # per-core degree sorts for both systems
    posA = np.empty(N, np.int64)   # node -> acc/table position (A order)
    posB = np.empty(N, np.int64)
    piA_all = []
    degA_sorted, degB_sorted = [], []
    for c in range(M):
        mask = dcore == c
        la = dloc[mask & is_a]
        lb = dloc[mask & ~is_a]
        degA = np.bincount(la, minlength=NPC)
        degB = np.bincount(lb, minlength=NPC)
        piA = np.argsort(-degA, kind="stable")
        piB = np.argsort(-degB, kind="stable")
        pA = np.empty(NPC, np.int64); pA[piA] = np.arange(NPC)
        pB = np.empty(NPC, np.int64); pB[piB] = np.arange(NPC)
        posA[c * NPC:(c + 1) * NPC] = pA
        posB[c * NPC:(c + 1) * NPC] = pB
        piA_all.append(piA)
        degA_sorted.append(degA[piA])
        degB_sorted.append(degB[piB])

    widthsA = _plan_rounds(degA_sorted)
    widthsB = _plan_rounds(degB_sorted)
    LA = int(np.sum(widthsA))
    LB = int(np.sum(widthsB))
    chunksA = _chunk_plan(widthsA, LA)
    chunksB = _chunk_plan(widthsB, LB)

    # global table row of a node (wrapped within its owner's slice)
    table_row = (np.arange(N) // NPC) * SLICE + _rowof(posA)

    in_maps = []
    for c in range(M):
        mask = dcore == c
        mA = mask & is_a
        mB = mask & ~is_a
        idxA, ewA = _build_system(posA[dst[mA]] , table_row[src[mA]], ew[mA], widthsA)
        idxB, ewB = _build_system(posB[dst[mB]], table_row[src[mB]] - HALF, ew[mB], widthsB)
        assert idxA.max(initial=0) < HALF and idxB.max(initial=0) < HALF

        # permute map: A-position q -> wrapped bounce row of the same node's
        # B-position. Pad positions (>= NPC) point at an always-zero row.
        piA = piA_all[c]
        rho = np.full(SLICE, NPC, np.int64)
        rho[:NPC] = posB[c * NPC + piA]
        rho_rows = _rowof(rho)

        # x slice, transposed, in A order (pad columns zero)
        xT = np.zeros((IN_DIM, SLICE), np.float32)
        xT[:, :NPC] = x[c * NPC + piA, :].T

        in_maps.append({
            "xT": xT,
            "idxA": _wrap16(idxA), "ewA": _wrap128(ewA),
            "idxB": _wrap16(idxB), "ewB": _wrap128(ewB),
            "rho": _wrap16(rho_rows),
        })

    plan = {
        "LA": LA, "LB": LB,
        "chunksA": chunksA, "chunksB": chunksB,
        "posA": posA,
    }
    return plan, in_maps


def _build(plan, W1, b1, W2, b2, W3, b3, Wlin, blin):
    import concourse.bacc as bacc
    import concourse.mybir as mybir
    import concourse.tile as tile

    LA, LB = plan["LA"], plan["LB"]
    f32 = mybir.dt.float32
    i16 = mybir.dt.int16

    nc = bacc.Bacc("TRN2", target_bir_lowering=False, debug=False,
                   num_devices=M, num_swdge_queues=NQ)

    # ---- I/O ----
    xT_d = nc.dram_tensor("xT", [IN_DIM, SLICE], f32, kind="ExternalInput")
    idxA_d = nc.dram_tensor("idxA", [128, LA // 16], i16, kind="ExternalInput")
    ewA_d = nc.dram_tensor("ewA", [128, LA // 128], f32, kind="ExternalInput")
    idxB_d = nc.dram_tensor("idxB", [128, LB // 16], i16, kind="ExternalInput")
    ewB_d = nc.dram_tensor("ewB", [128, LB // 128], f32, kind="ExternalInput")
    rho_d = nc.dram_tensor("rho", [128, SLICE // 16], i16, kind="ExternalInput")
    W1_d = nc.dram_tensor("W1", [IN_DIM, HID], f32, kind="ExternalInput")
    W2_d = nc.dram_tensor("W2", [HID, HID], f32, kind="ExternalInput")
    W3_d = nc.dram_tensor("W3", [128, HID], f32, kind="ExternalInput")  # rows 64-127 hold W3
    Wl12_d = nc.dram_tensor("Wl12", [128, OUT_DIM], f32, kind="ExternalInput")
    Wl3_d = nc.dram_tensor("Wl3", [HID, OUT_DIM], f32, kind="ExternalInput")
    bias_d = nc.dram_tensor("bias", [128, 3 * HID], f32, kind="ExternalInput")
    blin_d = nc.dram_tensor("blin", [128, OUT_DIM], f32, kind="ExternalInput")
    out_d = nc.dram_tensor("out", [128, SLOTS, OUT_DIM], f32, kind="ExternalOutput")

    # internal DRAM
    slice_d = nc.dram_tensor("slice_hw", [128, SLOTS, HID], f32)
    table_d = nc.dram_tensor("table", [TABLE_ROWS, HID], f32, addr_space="Shared")
    bounce_d = nc.dram_tensor("bounce", [SLICE, HID], f32)

    qctr = [0]

    def nextq():
        q = qctr[0] % NQ
        qctr[0] += 1
        return q

    with tile.TileContext(nc) as tc:
        with (
            tc.tile_pool(name="const", bufs=1) as constp,
            tc.tile_pool(name="acc", bufs=1) as accp,
            tc.tile_pool(name="ht", bufs=1) as htp,
            tc.tile_pool(name="stag", bufs=1) as stagp,
            tc.tile_pool(name="msg", bufs=8) as msgp,
            tc.tile_pool(name="small", bufs=6) as smallp,
            tc.tile_pool(name="ps", bufs=3, space="PSUM") as psp,
            tc.tile_pool(name="pso", bufs=2, space="PSUM") as psop,
        ):
            # ---- load constants ----
            xT = constp.tile([IN_DIM, SLICE], f32)
            idxA = constp.tile([128, LA // 16], i16)
            ewA = constp.tile([128, LA // 128], f32)
            idxB = constp.tile([128, LB // 16], i16)
            ewB = constp.tile([128, LB // 128], f32)
            rho = constp.tile([128, SLICE // 16], i16)
            W1t = constp.tile([IN_DIM, HID], f32)
            W2t = constp.tile([HID, HID], f32)
            W3t = constp.tile([128, HID], f32)  # W3 lives in partitions 64-127
            Wl12t = constp.tile([128, OUT_DIM], f32)
            Wl3t = constp.tile([HID, OUT_DIM], f32)
            biast = constp.tile([128, 3 * HID], f32)
            blint = constp.tile([128, OUT_DIM], f32)
            ident = constp.tile([128, 128], f32)

            for t, d in ((xT, xT_d), (idxA, idxA_d), (ewA, ewA_d),
                         (idxB, idxB_d), (ewB, ewB_d), (rho, rho_d),
                         (W1t, W1_d), (W2t, W2_d),                          (Wl12t, Wl12_d), (Wl3t, Wl3_d),
                         (biast, bias_d), (blint, blin_d)):
                nc.sync.dma_start(t[:], d[:])
            nc.sync.dma_start(W3t[:], W3_d[:])
            from concourse.masks import make_identity
            make_identity(nc, ident[:])

            h12T = htp.tile([128, SLICE], f32)   # rows 0-63: h1^T, 64-127: h2^T
            h3T = htp.tile([HID, SLICE], f32)

            relu = mybir.ActivationFunctionType.Relu
            copyf = mybir.ActivationFunctionType.Copy

            # ---- layer-1 input matmuls: slice of x @ W1 ----
            stag = stagp.tile([128, SLOTS, HID], f32, tag="stag")
            for m in range(SLOTS):
                ps = psp.tile([128, HID], f32, tag="mm")
                nc.tensor.matmul(ps[:], xT[:, m * 128:(m + 1) * 128], W1t[:],
                                 start=True, stop=True)
                nc.vector.tensor_copy(stag[:, m, :], ps[:])
            nc.sync.dma_start(slice_d[:], stag[:])

            ostag = stagp.tile([128, SLOTS, OUT_DIM], f32, tag="ostag")

            for layer in range(3):
                # ---- AllGather the table for this layer ----
                nc.gpsimd.collective_compute(
                    "AllGather", mybir.AluOpType.bypass,
                    replica_groups=[list(range(M))],
                    ins=[slice_d[:]], outs=[table_d[:]],
                )

                accA = accp.tile([128, SLOTS, HID], f32, tag="accA")
                accB = accp.tile([128, SLOTS, HID], f32, tag="accB")
                nc.vector.memset(accA[:], 0.0)
                nc.vector.memset(accB[:], 0.0)
                if layer > 0:
                    # tiny warmup gathers on each queue while the AllGather
                    # runs, so post-collective DGE state reload happens off
                    # the critical path (results are discarded)
                    for _ in range(NQ):
                        wmsg = msgp.tile([128, CMAX // 128, HID], f32, tag="msg")
                        nc.gpsimd.dma_gather(
                            wmsg[:, :1, :], bounce_d[:], rho[:, 0:8],
                            128, 128, HID, single_packet=False,
                            queue_num=nextq())

                def emit_chunks(acc, idx_t, ew_t, chunks, tbl):
                    for (off, w, segs) in chunks:
                        ws = w // 128
                        msg = msgp.tile([128, CMAX // 128, HID], f32, tag="msg")
                        nc.gpsimd.dma_gather(
                            msg[:, :ws, :], tbl, idx_t[:, off // 16:(off + w) // 16],
                            w, w, HID, single_packet=False, queue_num=nextq())
                        nc.vector.tensor_mul(
                            msg[:, :ws, :], msg[:, :ws, :],
                            ew_t[:, off // 128:(off + w) // 128]
                            .to_broadcast([128, ws, HID]))
                        for (ms, as_, ns) in segs:
                            nc.vector.tensor_add(
                                acc[:, as_:as_ + ns, :], acc[:, as_:as_ + ns, :],
                                msg[:, ms:ms + ns, :])

                # B system first so its bounce DMA overlaps the A gathers
                emit_chunks(accB, idxB, ewB, plan["chunksB"],
                            table_d[HALF:TABLE_ROWS, :])
                nc.sync.dma_start(
                    bounce_d[:].rearrange("(p s) d -> p s d", p=128), accB[:])
                chA = plan["chunksA"]
                emit_chunks(accA, idxA, ewA, chA[:-2], table_d[0:HALF, :])

                # permute-fold accB into accA (reads bounce written above);
                # interleaved before the tail A chunks to stay off the
                # critical path
                for off in range(0, SLICE, CMAX):
                    w = min(CMAX, SLICE - off)
                    ws = w // 128
                    msg = msgp.tile([128, CMAX // 128, HID], f32, tag="msg")
                    nc.gpsimd.dma_gather(
                        msg[:, :ws, :], bounce_d[:],
                        rho[:, off // 16:(off + w) // 16],
                        w, w, HID, single_packet=False, queue_num=nextq())
                    nc.vector.tensor_add(
                        accA[:, off // 128:off // 128 + ws, :],
                        accA[:, off // 128:off // 128 + ws, :], msg[:, :ws, :])
                emit_chunks(accA, idxA, ewA, chA[-2:], table_d[0:HALF, :])

                # ---- fused tail: bias+relu (batched), transpose, next matmul ----
                bslice = biast[:, layer * HID:(layer + 1) * HID]
                nc.vector.tensor_add(
                    accA[:], accA[:],
                    bslice.rearrange("p (s d) -> p s d", s=1)
                    .to_broadcast([128, SLOTS, HID]))
                nc.scalar.activation(accA[:], accA[:], relu)
                if layer < 2:
                    stag = stagp.tile([128, SLOTS, HID], f32, tag="stag")
                # phase 1: all transposes into h^T (PE + DVE pipeline)
                for m in range(SLOTS):
                    pst = psp.tile([HID, 128], f32, tag="tr")
                    nc.tensor.transpose(pst[:], accA[:, m, :], ident[:])
                    sl = slice(m * 128, (m + 1) * 128)
                    if layer == 0:
                        nc.vector.tensor_copy(h12T[0:HID, sl], pst[:])
                    elif layer == 1:
                        nc.vector.tensor_copy(h12T[HID:128, sl], pst[:])
                    else:
                        nc.vector.tensor_copy(h3T[:, sl], pst[:])
                # phase 2: all matmuls (lhsT already in SBUF)
                for m in range(SLOTS):
                    sl = slice(m * 128, (m + 1) * 128)
                    if layer == 0:
                        ps = psp.tile([128, HID], f32, tag="mm")
                        nc.tensor.matmul(ps[:], h12T[0:HID, sl], W2t[:],
                                         start=True, stop=True)
                        nc.vector.tensor_copy(stag[:, m, :], ps[:])
                    elif layer == 1:
                        ps = psp.tile([128, HID], f32, tag="mm")
                        nc.tensor.matmul(ps[:], h12T[HID:128, sl], W3t[HID:128, :],
                                         start=True, stop=True)
                        nc.vector.tensor_copy(stag[:, m, :], ps[:])
                    else:
                        pso = psop.tile([128, OUT_DIM], f32, tag="out")
                        nc.tensor.matmul(pso[:], h12T[:, sl],
                                         Wl12t[:], start=True, stop=False)
                        nc.tensor.matmul(pso[:], h3T[:, sl],
                                         Wl3t[:], start=False, stop=True)
                        nc.vector.tensor_add(ostag[:, m, :], pso[:], blint[:])
                if layer < 2:
                    nc.sync.dma_start(slice_d[:], stag[:])

            nc.sync.dma_start(out_d[:], ostag[:])

    nc.compile()
    return nc


_CACHE = {}


def kernel(x, edge_index, edge_weight, W1, b1, W2, b2, W3, b3, Wlin, blin):
    from concourse.bass_utils import run_bass_kernel_spmd

    x = np.asarray(x, dtype=np.float32)
    assert x.shape == (N, IN_DIM) and np.asarray(edge_index).shape == (2, E)

    key = hash(np.asarray(edge_index).tobytes())
    if key not in _CACHE:
        plan, in_maps = _prep(x, edge_index, edge_weight)
        nc = _build(plan, W1, b1, W2, b2, W3, b3, Wlin, blin)
        _CACHE[key] = (plan, nc)
    else:
        plan, nc = _CACHE[key]
        _, in_maps = _prep(x, edge_index, edge_weight)

    Wlin = np.asarray(Wlin, dtype=np.float32)
    shared = {
        "W1": np.asarray(W1, np.float32), "W2": np.asarray(W2, np.float32),
        "W3": np.concatenate([np.zeros((HID, HID), np.float32), np.asarray(W3, np.float32)], axis=0),
        "Wl12": np.ascontiguousarray(Wlin[0:128]), "Wl3": np.ascontiguousarray(Wlin[128:192]),
        "bias": np.tile(np.concatenate([np.asarray(b, np.float32) for b in (b1, b2, b3)])[None, :], (128, 1)),
        "blin": np.tile(np.asarray(blin, np.float32)[None, :], (128, 1)),
    }
    for im in in_maps:
        im.update(shared)

    res = run_bass_kernel_spmd(nc, in_maps, core_ids=list(range(M)))
    kernel._last_results = res
    kernel._last_in_maps = in_maps
    kernel._last_nc = nc

    posA = plan["posA"]
    out = np.empty((N, OUT_DIM), np.float32)
    for c in range(M):
        oc = res.results[c]["out"]  # [128, SLOTS, OUT]
        q = posA[c * NPC:(c + 1) * NPC]
        out[c * NPC:(c + 1) * NPC] = oc[q % 128, q // 128, :]
    return out


# revision 19
# speedup vs baseline: 1.0498x; 1.0498x over previous
"""Trainium2 Bass kernel for a 3-layer GCN (JKNet, mode='cat') — 8-core SPMD.

Strategy (dst-sharded graph parallelism):
  - Nodes are partitioned across 8 cores (6250 each, padded to 6272 = 49*128).
    Each core owns all edges whose destination lands in its range.
  - Per layer: each core computes its slice of h @ W, an AllGather builds the
    full 50176x64 f32 gather table in DRAM, then the core gathers hW[src] for
    its edges with GPSIMD dma_gather (4 SWDGE queues), scales by edge weight
    on DVE and accumulates into an SBUF accumulator using a "rounds" schedule:
    nodes are degree-sorted so round s covers a dense prefix of node slots,
    making the accumulate a plain strided DVE add (no scatter anywhere).
  - dma_gather indices are int16, so the table is split in two halves
    (rows < 25088 / >= 25088, i.e. src cores 0-3 vs 4-7). Each half gets its
    own degree-sort permutation and accumulator; the second accumulator is
    reconciled through a DRAM bounce + local permute-gather.
  - Bias + ReLU on DVE/ACT, PE transpose produces h^T for the next layer's
    matmul and for the final JumpingKnowledge concat matmul.

Self-contained: hardcodes the problem geometry (N=50000, E=800000, 128->64,
3 layers, out 40) but computes all data-dependent schedules from the inputs.
"""

import sys

sys.path.insert(0, "/opt/trn_rl_repo")

import numpy as np

N = 50000
E = 800000
IN_DIM = 128
HID = 64
OUT_DIM = 40
M = 8               # cores
NPC = N // M        # 6250 nodes per core
SLOTS = 49          # ceil(6250/128)
SLICE = SLOTS * 128  # 6272 padded rows per core slice
TABLE_ROWS = M * SLICE  # 50176
HALF = 4 * SLICE    # 25088 (int16-safe boundary; src cores 0-3 vs 4-7)
CMAX = 1024         # max indices per dma_gather instruction
NQ = 4              # SWDGE queues


def _wrap16(a):
    """Flat [L] -> [128, L//16] int16, index j at partition j%16, slot j//16,
    replicated across the 8 GPSIMD core groups."""
    L = a.shape[0]
    return np.tile(a.reshape(L // 16, 16).T, (8, 1)).astype(np.int16)


def _wrap128(a):
    """Flat [L] -> [128, L//128], position j at partition j%128, slot j//128."""
    L = a.shape[0]
    return np.ascontiguousarray(a.reshape(L // 128, 128).T)


def _rowof(q):
    """acc position q -> wrapped DRAM row index (partition-major layout)."""
    return (q % 128) * SLOTS + q // 128


def _ranks_within(p):
    """For int array p, rank of each element among equal values (stable)."""
    order = np.argsort(p, kind="stable")
    ps = p[order]
    starts = np.r_[0, np.nonzero(np.diff(ps))[0] + 1]
    counts = np.diff(np.r_[starts, len(ps)])
    r_sorted = np.arange(len(ps)) - np.repeat(starts, counts)
    r = np.empty_like(r_sorted)
    r[order] = r_sorted
    return r


def _build_system(pos_of_dst, table_row_of_src, ew, max_rounds_widths):
    """Build flat (idx, ew) arrays for one round-system of one core.

    pos_of_dst: per-edge acc position of the destination node (degree-sorted).
    table_row_of_src: per-edge gather index (already half-relative).
    max_rounds_widths: global per-round padded widths W_s (list, multiple of 128).
    Returns (idx_flat int64, ew_flat f32).
    """
    roundoff = np.r_[0, np.cumsum(max_rounds_widths)]
    L = int(roundoff[-1])
    idx_flat = np.zeros(L, np.int64)
    ew_flat = np.zeros(L, np.float32)
    r = _ranks_within(pos_of_dst)
    flatpos = roundoff[r] + pos_of_dst
    idx_flat[flatpos] = table_row_of_src
    ew_flat[flatpos] = ew
    return idx_flat, ew_flat


def _plan_rounds(deg_by_core):
    """deg_by_core: [M, NPC] degree of each node (in its own sort order, desc).
    Returns list of global padded round widths (each a multiple of 128)."""
    smax = int(max(d[0] for d in deg_by_core)) if len(deg_by_core) else 0
    widths = []
    for s in range(smax):
        n_s = max(int((d > s).sum()) for d in deg_by_core)
        if n_s == 0:
            break
        widths.append(((n_s + 127) // 128) * 128)
    return widths


def _chunk_plan(widths, total_pad128):
    """Split flat [0, L) (concatenated padded rounds) into gather chunks of at
    most CMAX (multiples of 128), with per-chunk accumulate segments.
    Returns list of (off, w, [(msg_slot0, acc_slot0, nslots), ...])."""
    roundoff = np.r_[0, np.cumsum(widths)].astype(np.int64)
    L = int(roundoff[-1])
    assert L == total_pad128
    chunks = []
    off = 0
    while off < L:
        w = min(CMAX, L - off)
        # segments: intersect [off, off+w) with rounds
        segs = []
        s = int(np.searchsorted(roundoff, off, side="right")) - 1
        a = off
        while a < off + w:
            b = min(off + w, int(roundoff[s + 1]))
            segs.append(((a - off) // 128, int(a - roundoff[s]) // 128, (b - a) // 128))
            a = b
            s += 1
        chunks.append((off, w, segs))
        off += w
    return chunks


def _prep(x, edge_index, edge_weight):
    """All host-side index prep. Returns (plan dict, per-core input maps)."""
    src = np.asarray(edge_index[0], dtype=np.int64)
    dst = np.asarray(edge_index[1], dtype=np.int64)
    ew = np.asarray(edge_weight, dtype=np.float32)
    x = np.asarray(x, dtype=np.float32)

    dcore = dst // NPC
    dloc = dst - dcore * NPC
    score = src // NPC
    is_a = score <= 3

    # per-core degree sorts for both systems
    posA = np.empty(N, np.int64)   # node -> acc/table position (A order)
    posB = np.empty(N, np.int64)
    piA_all = []
    degA_sorted, degB_sorted = [], []
    for c in range(M):
        mask = dcore == c
        la = dloc[mask & is_a]
        lb = dloc[mask & ~is_a]
        degA = np.bincount(la, minlength=NPC)
        degB = np.bincount(lb, minlength=NPC)
        piA = np.argsort(-degA, kind="stable")
        piB = np.argsort(-degB, kind="stable")
        pA = np.empty(NPC, np.int64); pA[piA] = np.arange(NPC)
        pB = np.empty(NPC, np.int64); pB[piB] = np.arange(NPC)
        posA[c * NPC:(c + 1) * NPC] = pA
        posB[c * NPC:(c + 1) * NPC] = pB
        piA_all.append(piA)
        degA_sorted.append(degA[piA])
        degB_sorted.append(degB[piB])

    widthsA = _plan_rounds(degA_sorted)
    widthsB = _plan_rounds(degB_sorted)
    LA = int(np.sum(widthsA))
    LB = int(np.sum(widthsB))
    chunksA = _chunk_plan(widthsA, LA)
    chunksB = _chunk_plan(widthsB, LB)

    # global table row of a node (wrapped within its owner's slice)
    table_row = (np.arange(N) // NPC) * SLICE + _rowof(posA)

    in_maps = []
    for c in range(M):
        mask = dcore == c
        mA = mask & is_a
        mB = mask & ~is_a
        idxA, ewA = _build_system(posA[dst[mA]] , table_row[src[mA]], ew[mA], widthsA)
        idxB, ewB = _build_system(posB[dst[mB]], table_row[src[mB]] - HALF, ew[mB], widthsB)
        assert idxA.max(initial=0) < HALF and idxB.max(initial=0) < HALF

        # permute map: A-position q -> wrapped bounce row of the same node's
        # B-position. Pad positions (>= NPC) point at an always-zero row.
        piA = piA_all[c]
        rho = np.full(SLICE, NPC, np.int64)
        rho[:NPC] = posB[c * NPC + piA]
        rho_rows = _rowof(rho)

        # x slice, transposed, in A order (pad columns zero)
        xT = np.zeros((IN_DIM, SLICE), np.float32)
        xT[:, :NPC] = x[c * NPC + piA, :].T

        in_maps.append({
            "xT": xT,
            "idxA": _wrap16(idxA), "ewA": _wrap128(ewA),
            "idxB": _wrap16(idxB), "ewB": _wrap128(ewB),
            "rho": _wrap16(rho_rows),
        })

    plan = {
        "LA": LA, "LB": LB,
        "chunksA": chunksA, "chunksB": chunksB,
        "posA": posA,
    }
    return plan, in_maps


def _build(plan, W1, b1, W2, b2, W3, b3, Wlin, blin):
    import concourse.bacc as bacc
    import concourse.mybir as mybir
    import concourse.tile as tile

    LA, LB = plan["LA"], plan["LB"]
    f32 = mybir.dt.float32
    i16 = mybir.dt.int16

    nc = bacc.Bacc("TRN2", target_bir_lowering=False, debug=False,
                   num_devices=M, num_swdge_queues=NQ)

    # ---- I/O ----
    xT_d = nc.dram_tensor("xT", [IN_DIM, SLICE], f32, kind="ExternalInput")
    idxA_d = nc.dram_tensor("idxA", [128, LA // 16], i16, kind="ExternalInput")
    ewA_d = nc.dram_tensor("ewA", [128, LA // 128], f32, kind="ExternalInput")
    idxB_d = nc.dram_tensor("idxB", [128, LB // 16], i16, kind="ExternalInput")
    ewB_d = nc.dram_tensor("ewB", [128, LB // 128], f32, kind="ExternalInput")
    rho_d = nc.dram_tensor("rho", [128, SLICE // 16], i16, kind="ExternalInput")
    W1_d = nc.dram_tensor("W1", [IN_DIM, HID], f32, kind="ExternalInput")
    W2_d = nc.dram_tensor("W2", [HID, HID], f32, kind="ExternalInput")
    W3_d = nc.dram_tensor("W3", [128, HID], f32, kind="ExternalInput")  # rows 64-127 hold W3
    Wl12_d = nc.dram_tensor("Wl12", [128, OUT_DIM], f32, kind="ExternalInput")
    Wl3_d = nc.dram_tensor("Wl3", [HID, OUT_DIM], f32, kind="ExternalInput")
    bias_d = nc.dram_tensor("bias", [128, 3 * HID], f32, kind="ExternalInput")
    blin_d = nc.dram_tensor("blin", [128, OUT_DIM], f32, kind="ExternalInput")
    out_d = nc.dram_tensor("out", [128, SLOTS, OUT_DIM], f32, kind="ExternalOutput")

    # internal DRAM
    slice_d = nc.dram_tensor("slice_hw", [128, SLOTS, HID], f32)
    table_d = nc.dram_tensor("table", [TABLE_ROWS, HID], f32, addr_space="Shared")
    bounce_d = nc.dram_tensor("bounce", [SLICE, HID], f32)

    qctr = [0]

    def nextq():
        q = qctr[0] % NQ
        qctr[0] += 1
        return q

    with tile.TileContext(nc) as tc:
        with (
            tc.tile_pool(name="const", bufs=1) as constp,
            tc.tile_pool(name="acc", bufs=1) as accp,
            tc.tile_pool(name="ht", bufs=1) as htp,
            tc.tile_pool(name="stag", bufs=1) as stagp,
            tc.tile_pool(name="msg", bufs=16) as msgp,
            tc.tile_pool(name="small", bufs=6) as smallp,
            tc.tile_pool(name="ps", bufs=3, space="PSUM") as psp,
            tc.tile_pool(name="pso", bufs=2, space="PSUM") as psop,
        ):
            # ---- load constants ----
            xT = constp.tile([IN_DIM, SLICE], f32)
            idxA = constp.tile([128, LA // 16], i16)
            ewA = constp.tile([128, LA // 128], f32)
            idxB = constp.tile([128, LB // 16], i16)
            ewB = constp.tile([128, LB // 128], f32)
            rho = constp.tile([128, SLICE // 16], i16)
            W1t = constp.tile([IN_DIM, HID], f32)
            W2t = constp.tile([HID, HID], f32)
            W3t = constp.tile([128, HID], f32)  # W3 lives in partitions 64-127
            Wl12t = constp.tile([128, OUT_DIM], f32)
            Wl3t = constp.tile([HID, OUT_DIM], f32)
            biast = constp.tile([128, 3 * HID], f32)
            blint = constp.tile([128, OUT_DIM], f32)
            ident = constp.tile([128, 128], f32)

            for t, d in ((xT, xT_d), (idxA, idxA_d), (ewA, ewA_d),
                         (idxB, idxB_d), (ewB, ewB_d), (rho, rho_d),
                         (W1t, W1_d), (W2t, W2_d),                          (Wl12t, Wl12_d), (Wl3t, Wl3_d),
                         (biast, bias_d), (blint, blin_d)):
                nc.sync.dma_start(t[:], d[:])
            nc.sync.dma_start(W3t[:], W3_d[:])
            from concourse.masks import make_identity
            make_identity(nc, ident[:])

            h12T = htp.tile([128, SLICE], f32)   # rows 0-63: h1^T, 64-127: h2^T
            h3T = htp.tile([HID, SLICE], f32)

            relu = mybir.ActivationFunctionType.Relu
            copyf = mybir.ActivationFunctionType.Copy

            # ---- layer-1 input matmuls: slice of x @ W1 ----
            stag = stagp.tile([128, SLOTS, HID], f32, tag="stag")
            for m in range(SLOTS):
                ps = psp.tile([128, HID], f32, tag="mm")
                nc.tensor.matmul(ps[:], xT[:, m * 128:(m + 1) * 128], W1t[:],
                                 start=True, stop=True)
                nc.vector.tensor_copy(stag[:, m, :], ps[:])
            nc.sync.dma_start(slice_d[:], stag[:])

            ostag = stagp.tile([128, SLOTS, OUT_DIM], f32, tag="ostag")

            for layer in range(3):
                # ---- AllGather the table for this layer ----
                nc.gpsimd.collective_compute(
                    "AllGather", mybir.AluOpType.bypass,
                    replica_groups=[list(range(M))],
                    ins=[slice_d[:]], outs=[table_d[:]],
                )

                accA = accp.tile([128, SLOTS, HID], f32, tag="accA")
                accB = accp.tile([128, SLOTS, HID], f32, tag="accB")
                nc.vector.memset(accA[:], 0.0)
                nc.vector.memset(accB[:], 0.0)
                if layer > 0:
                    # tiny warmup gathers on each queue while the AllGather
                    # runs, so post-collective DGE state reload happens off
                    # the critical path (results are discarded)
                    for _ in range(NQ):
                        wmsg = msgp.tile([128, CMAX // 128, HID], f32, tag="msg")
                        nc.gpsimd.dma_gather(
                            wmsg[:, :1, :], bounce_d[:], rho[:, 0:8],
                            128, 128, HID, single_packet=False,
                            queue_num=nextq())

                def emit_chunks(acc, idx_t, ew_t, chunks, tbl):
                    for (off, w, segs) in chunks:
                        ws = w // 128
                        msg = msgp.tile([128, CMAX // 128, HID], f32, tag="msg")
                        nc.gpsimd.dma_gather(
                            msg[:, :ws, :], tbl, idx_t[:, off // 16:(off + w) // 16],
                            w, w, HID, single_packet=False, queue_num=nextq())
                        nc.vector.tensor_mul(
                            msg[:, :ws, :], msg[:, :ws, :],
                            ew_t[:, off // 128:(off + w) // 128]
                            .to_broadcast([128, ws, HID]))
                        for (ms, as_, ns) in segs:
                            nc.vector.tensor_add(
                                acc[:, as_:as_ + ns, :], acc[:, as_:as_ + ns, :],
                                msg[:, ms:ms + ns, :])

                # B system first so its bounce DMA overlaps the A gathers
                emit_chunks(accB, idxB, ewB, plan["chunksB"],
                            table_d[HALF:TABLE_ROWS, :])
                nc.sync.dma_start(
                    bounce_d[:].rearrange("(p s) d -> p s d", p=128), accB[:])
                chA = plan["chunksA"]
                emit_chunks(accA, idxA, ewA, chA[:-2], table_d[0:HALF, :])

                # permute-fold accB into accA (reads bounce written above);
                # interleaved before the tail A chunks to stay off the
                # critical path
                for off in range(0, SLICE, CMAX):
                    w = min(CMAX, SLICE - off)
                    ws = w // 128
                    msg = msgp.tile([128, CMAX // 128, HID], f32, tag="msg")
                    nc.gpsimd.dma_gather(
                        msg[:, :ws, :], bounce_d[:],
                        rho[:, off // 16:(off + w) // 16],
                        w, w, HID, single_packet=False, queue_num=nextq())
                    nc.vector.tensor_add(
                        accA[:, off // 128:off // 128 + ws, :],
                        accA[:, off // 128:off // 128 + ws, :], msg[:, :ws, :])
                emit_chunks(accA, idxA, ewA, chA[-2:], table_d[0:HALF, :])

                # ---- fused tail: bias+relu (batched), transpose, next matmul ----
                bslice = biast[:, layer * HID:(layer + 1) * HID]
                nc.vector.tensor_add(
                    accA[:], accA[:],
                    bslice.rearrange("p (s d) -> p s d", s=1)
                    .to_broadcast([128, SLOTS, HID]))
                nc.scalar.activation(accA[:], accA[:], relu)
                if layer < 2:
                    stag = stagp.tile([128, SLOTS, HID], f32, tag="stag")
                # phase 1: all transposes into h^T (PE + DVE pipeline)
                for m in range(SLOTS):
                    pst = psp.tile([HID, 128], f32, tag="tr")
                    nc.tensor.transpose(pst[:], accA[:, m, :], ident[:])
                    sl = slice(m * 128, (m + 1) * 128)
                    if layer == 0:
                        nc.vector.tensor_copy(h12T[0:HID, sl], pst[:])
                    elif layer == 1:
                        nc.vector.tensor_copy(h12T[HID:128, sl], pst[:])
                    else:
                        nc.vector.tensor_copy(h3T[:, sl], pst[:])
                # phase 2: all matmuls (lhsT already in SBUF)
                for m in range(SLOTS):
                    sl = slice(m * 128, (m + 1) * 128)
                    if layer == 0:
                        ps = psp.tile([128, HID], f32, tag="mm")
                        nc.tensor.matmul(ps[:], h12T[0:HID, sl], W2t[:],
                                         start=True, stop=True)
                        nc.vector.tensor_copy(stag[:, m, :], ps[:])
                    elif layer == 1:
                        ps = psp.tile([128, HID], f32, tag="mm")
                        nc.tensor.matmul(ps[:], h12T[HID:128, sl], W3t[HID:128, :],
                                         start=True, stop=True)
                        nc.vector.tensor_copy(stag[:, m, :], ps[:])
                    else:
                        pso = psop.tile([128, OUT_DIM], f32, tag="out")
                        nc.tensor.matmul(pso[:], h12T[:, sl],
                                         Wl12t[:], start=True, stop=False)
                        nc.tensor.matmul(pso[:], h3T[:, sl],
                                         Wl3t[:], start=False, stop=True)
                        nc.vector.tensor_add(ostag[:, m, :], pso[:], blint[:])
                if layer < 2:
                    nc.sync.dma_start(slice_d[:], stag[:])

            nc.sync.dma_start(out_d[:], ostag[:])

    nc.compile()
    return nc


_CACHE = {}


def kernel(x, edge_index, edge_weight, W1, b1, W2, b2, W3, b3, Wlin, blin):
    from concourse.bass_utils import run_bass_kernel_spmd

    x = np.asarray(x, dtype=np.float32)
    assert x.shape == (N, IN_DIM) and np.asarray(edge_index).shape == (2, E)

    key = hash(np.asarray(edge_index).tobytes())
    if key not in _CACHE:
        plan, in_maps = _prep(x, edge_index, edge_weight)
        nc = _build(plan, W1, b1, W2, b2, W3, b3, Wlin, blin)
        _CACHE[key] = (plan, nc)
    else:
        plan, nc = _CACHE[key]
        _, in_maps = _prep(x, edge_index, edge_weight)

    Wlin = np.asarray(Wlin, dtype=np.float32)
    shared = {
        "W1": np.asarray(W1, np.float32), "W2": np.asarray(W2, np.float32),
        "W3": np.concatenate([np.zeros((HID, HID), np.float32), np.asarray(W3, np.float32)], axis=0),
        "Wl12": np.ascontiguousarray(Wlin[0:128]), "Wl3": np.ascontiguousarray(Wlin[128:192]),
        "bias": np.tile(np.concatenate([np.asarray(b, np.float32) for b in (b1, b2, b3)])[None, :], (128, 1)),
        "blin": np.tile(np.asarray(blin, np.float32)[None, :], (128, 1)),
    }
    for im in in_maps:
        im.update(shared)

    res = run_bass_kernel_spmd(nc, in_maps, core_ids=list(range(M)))
    kernel._last_results = res
    kernel._last_in_maps = in_maps
    kernel._last_nc = nc

    posA = plan["posA"]
    out = np.empty((N, OUT_DIM), np.float32)
    for c in range(M):
        oc = res.results[c]["out"]  # [128, SLOTS, OUT]
        q = posA[c * NPC:(c + 1) * NPC]
        out[c * NPC:(c + 1) * NPC] = oc[q % 128, q // 128, :]
    return out
